# revision 19
# baseline (speedup 1.0000x reference)
"""Trainium2 Bass kernel for bag-level attention (ragged_sequence).

Math (per bag b over its 16 sentences i):
    att_i  = <x_i, rel[q_i]>
    w      = softmax(att) within bag
    logits = (sum_i w_i x_i) @ rel.T + bias

Key identity: logits[b] = sum_i w_i S[i,:] + bias with S = x @ rel.T, so x is
read from HBM exactly once. x and rel travel as fp16 (gate is rel_err<2e-2;
fp16 end-to-end lands ~1e-3). The contraction over D=768 is split into 6
chunks of 128; chunks 0-2 accumulate on PE column-tile (0,0) into PSUM rows
0:64, chunks 3-5 on tile (0,64) into rows 64:128, so the full score of class c
for sentence j is st[c,j] + st[75+c,j] (block-2 classes sit at rows 75:128,
leaving st row 64 zero for the z-selector trick).

The 32768 sentences per core are processed as a list of column PIECES: 4x256
at the start (so the first matmul starts ~1.3us after launch instead of
waiting for a 1.57MB transfer), 30x1024 in the middle (12KB-per-partition
contiguous DMA runs that saturate the SDMA queues), 4x256 at the end (so the
post-DMA pipeline drain flushes in short waves). x is packed on the host so
each piece is ONE contiguous run per partition. Piece DMAs alternate between
the SP and ACT HWDGE rings so one ring's completion latency hides behind the
other ring's transfer and the SDMA queues never idle between pieces.

Device pipeline per piece, software-pipelined with a deep skew — stage A at
piece i, stage B at i-2, stage C at i-4 — so every engine always has runnable
work and the softmax latency chain (5 engine hops) never gates the next
piece's matmuls:
  A:  st   = x @ rel.T              (PE, fp16 matmuls, PSUM fp32)
      sc16 = (st + sel64) fp16      (ACT copy PSUM->SBUF with sel64 as the
             free per-partition bias; this copy is what breaks the st-PSUM
             lifetime so st needs only 2 PSUM bufs despite the deep skew)
      sm   = st * onehot(q)         (DVE, PSUM fp32 x fp8 -> SBUF fp16)
  B:  att  = ones128.T @ sm         (PE, [1,<=512] matmuls)
      e    = exp(att - 4*ln2)       (ACT; the -4ln2 bias scales e by 1/16 so
             the weighted sums below stay inside fp16 range; it divides out
             of the final softmax normalize exactly)
      ebs  = partition_broadcast(e) (GpSimd, 512-wide halves)
  C:  w    = sc16 * ebs             (DVE, fp16 SBUF 2x mode; w row 64 is e)
      lu   = windowed reduce_16(w)  (DVE) -> [128, bags] fp16
      lc[:, bag slice] = sident.T @ lu  (PE) accumulated into a per-slab
             PSUM tile [128, 512]; sident folds block recombine + bias*z and
             extracts z into row 64
Per 512-bag slab: ACT copies lc rows 0:65 PSUM->SBUF fp16 and DMAs them out
via the ACT HWDGE ring. The final divide by z (row 64) happens on the host.

Output is stored transposed [65, bags]; host divides and transposes back.
"""

import os
from contextlib import ExitStack

import ml_dtypes
import numpy as np

import concourse.bass as bass
import concourse.tile as tile
from concourse import bacc, library_config, mybir
from concourse.bass_utils import run_bass_kernel_spmd

# Problem constants (hardcoded per spec nn_Attention_85478439125349)
N = 262144
B = 16384
D = 768
C = 53
BAG = 16
N_CORES = 8
ROWS = N // N_CORES          # 32768 sentences per core
BAGS = B // N_CORES          # 2048 bags per core
KCH = D // 128               # 6 contraction chunks
F32 = mybir.dt.float32
F16 = mybir.dt.float16
F8 = mybir.dt.float8e4
EXP_BIAS = -2.772588722239781   # -4*ln2: e' = e/16, cancels in w = e'/z'
SLAB_BAGS = 512                 # bags per output slab (1 PSUM bank)
OH_GROUP = 1024                 # one-hot mask DMA granularity (columns)


def _pieces(rows: int):
    """Compute pieces: 1024-column body, 4x256 tail (short drain waves).
    DMA granularity stays a full 1024-column superchunk regardless."""
    ps = [1024] * (rows // 1024 - 1) + [256] * 4
    assert sum(ps) == rows
    out, off = [], 0
    for ln in ps:
        out.append((off, ln))
        off += ln
    return out


def build_nc(rows: int) -> bass.Bass:
    """Build the per-core Bass program for `rows` sentences (bags of BAG)."""
    bags = rows // BAG
    pieces = _pieces(rows)

    nc = bacc.Bacc()
    # x fp16 packed so each piece is one contiguous per-partition run of
    # KCH*len elements (k-major inside the piece)
    xt2 = nc.declare_dram_parameter("xt2", [128, rows * KCH], F16, isOutput=False)
    # one-hot mask replicated into both partition blocks: [128, rows], fp8
    oht = nc.declare_dram_parameter("oht", [128, rows], F8, isOutput=False)
    # relT packed for lhsT loads: relt[p, k, c] = rel[c, 128k+p], c pad to 64
    relt = nc.declare_dram_parameter("relt", [128, KCH, 64], F16, isOutput=False)
    # recombine matrix [128, 128] fp16: col c (<53) has 1.0 at rows c and 75+c
    # and bias_c at row 64; col 64 has 1.0 at row 64 (z extraction, placed at
    # 64 because engine APs may only start at partition 0/32/64/96); rest 0.
    sident = nc.declare_dram_parameter("sident", [128, 128], F16, isOutput=False)
    # selector column: 1.0 only in row 64 (puts e into w row 64 -> z in lu)
    sel64d = nc.declare_dram_parameter("sel64", [128, 1], F16, isOutput=False)
    # un-normalized logits (rows 0:53) + z (row 64), divided on host
    out65 = nc.declare_dram_parameter("out65", [65, bags], F16, isOutput=True)

    with tile.TileContext(nc) as tc, ExitStack() as ctx:
        consts = ctx.enter_context(tc.tile_pool(name="consts", bufs=1))
        xpool = ctx.enter_context(tc.tile_pool(name="xpool", bufs=8))
        ohpool = ctx.enter_context(tc.tile_pool(name="ohpool", bufs=8))
        work = ctx.enter_context(tc.tile_pool(name="work", bufs=3))
        psum = ctx.enter_context(tc.tile_pool(name="psum", bufs=2, space="PSUM"))

        # --- constants ---
        # const DMAs go on the ACT ring so the first x superchunk is
        # the very first instruction on the sync ring (saves startup time)
        relt_sb = consts.tile([128, KCH, 64], F16)
        nc.scalar.dma_start(out=relt_sb, in_=relt[:, :, :])
        sident_sb = consts.tile([128, 128], F16)
        nc.scalar.dma_start(out=sident_sb, in_=sident[:, :])
        ones128 = consts.tile([128, 1], F16)
        nc.vector.memset(ones128, 1.0)
        sel64 = consts.tile([128, 1], F16)
        nc.scalar.dma_start(out=sel64, in_=sel64d[:, :])
        ebias = consts.tile([128, 1], F32)
        nc.vector.memset(ebias, EXP_BIAS)
        nc.gpsimd.load_library(library_config.attn)

        pend_a = {}   # i -> (sc16, sm, ch)
        pend_b = {}   # i -> (sc16, ebs, ch)
        lc_sl = [None]   # current slab PSUM tile [128, SLAB_BAGS]
        oh_state = [None, -1, None]   # oht tile, group index, x 3D view

        def stage_b(i):
            sc16, sm, ch = pend_a.pop(i)
            e = work.tile([1, ch], F16, tag="e", bufs=3, name="e",
                          padded_shape=[1, 1024])
            ebs = work.tile([128, ch], F16, tag="ebs", bufs=4, name="ebs",
                            padded_shape=[128, 1024])
            for h0 in range(0, ch, 512):
                hw = min(512, ch - h0)
                hs = slice(h0, h0 + hw)
                attp = psum.tile([1, hw], F32, tag="att", bufs=2, name="attp",
                                 padded_shape=[1, 512])
                nc.tensor.matmul(attp, lhsT=ones128, rhs=sm[:, hs])
                nc.scalar.activation(
                    e[:, hs], attp, mybir.ActivationFunctionType.Exp,
                    bias=ebias[0:1, :],
                )
                nc.gpsimd.partition_broadcast(ebs[:, hs], e[:, hs], channels=128)
            pend_b[i] = (sc16, ebs, ch)

        def stage_c(i, bag_off):
            sc16, ebs, ch = pend_b.pop(i)
            chb = ch // BAG
            w = work.tile([128, ch], F16, tag="w", bufs=2, name="w",
                          padded_shape=[128, 1024])
            nc.vector.tensor_mul(w, sc16, ebs)
            lu = work.tile([128, chb], F16, tag="lu", bufs=3, name="lu",
                           padded_shape=[128, 64])
            with nc.allow_low_precision("fp16 bag sums stay < 2^14, rel 5e-4"):
                nc.vector.reduce_sum(
                    lu,
                    w.rearrange("p (b j) -> p b j", j=BAG),
                    axis=mybir.AxisListType.X,
                )
            ob = bag_off % SLAB_BAGS
            if ob == 0:
                lc_sl[0] = psum.tile(
                    [128, SLAB_BAGS], F32, tag="lc", bufs=2, name="lc"
                )
            # recombines the two partition blocks, folds bias*z into rows
            # 0:53 and extracts z into row 64
            nc.tensor.matmul(
                lc_sl[0][:, ob : ob + chb], lhsT=sident_sb, rhs=lu
            )
            if ob + chb == SLAB_BAGS:
                ltc = work.tile([65, SLAB_BAGS], F16, tag="ltc", bufs=2,
                                name="ltc")
                nc.scalar.copy(ltc, lc_sl[0][0:65, :])
                s0 = bag_off + chb - SLAB_BAGS
                # out DMA issued via the GpSimd SWDGE ring so its issue/wait
                # never blocks either HWDGE ring's x-prefetch stream
                nc.gpsimd.dma_start(out=out65[:, s0 : s0 + SLAB_BAGS], in_=ltc)

        n_total = len(pieces)
        bag_off_c = 0
        for i in range(n_total + 4):
            if i < n_total:
                gcol, ch = pieces[i]
                grp = gcol // OH_GROUP
                if grp != oh_state[1]:
                    oh_state[1] = grp
                    # alternate the two HWDGE rings (SP / ACT) per superchunk
                    # so one ring's completion latency hides behind the other
                    # ring's transfer and the SDMA queues never idle; DMA is
                    # always a full 1024-column superchunk (flat 2D->2D: one
                    # contiguous 12KB descriptor run per partition)
                    eng = nc.sync if grp % 2 == 0 else nc.scalar
                    oeng = nc.scalar if grp % 2 == 0 else nc.sync
                    x_fl = xpool.tile(
                        [128, KCH * OH_GROUP], F16, tag="x", bufs=8,
                        name="x_fl",
                    )
                    eng.dma_start(
                        out=x_fl,
                        in_=xt2[:, grp * OH_GROUP * KCH
                                : (grp + 1) * OH_GROUP * KCH],
                    )
                    oh_state[2] = x_fl.rearrange("p (k j) -> p k j", k=KCH)
                    oh_sb = ohpool.tile([128, OH_GROUP], F8, tag="oh", bufs=8,
                                        name="oh_sb")
                    oeng.dma_start(
                        out=oh_sb,
                        in_=oht[:, grp * OH_GROUP : (grp + 1) * OH_GROUP],
                    )
                    oh_state[0] = oh_sb
                lo = gcol - grp * OH_GROUP
                x_sb = oh_state[2][:, :, lo : lo + ch]
                cs = slice(lo, lo + ch)
                st = psum.tile([128, ch], F32, tag="st", bufs=2, name="st",
                               padded_shape=[128, 1024])
                # Each column-half runs its own start=True accumulation chain
                # ((0,64) emitted first). matmul moving dim is ISA-capped at
                # 512 columns, so each k-chunk is fed in <=512-wide halves.
                for h0 in range(0, ch, 512):
                    hw = min(512, ch - h0)
                    os_ = slice(h0, h0 + hw)
                    for k in range(KCH // 2, KCH):
                        nc.tensor.matmul(
                            st[64:128, os_],
                            lhsT=relt_sb[:, k, :],
                            rhs=x_sb[:, k, h0 : h0 + hw],
                            start=(k == KCH // 2),
                            stop=False,
                            skip_group_check=True,
                            tile_position=(0, 64),
                        )
                    for k in range(KCH // 2):
                        nc.tensor.matmul(
                            st[0:64, os_],
                            lhsT=relt_sb[:, k, :],
                            rhs=x_sb[:, k, h0 : h0 + hw],
                            start=(k == 0),
                            stop=(k == KCH // 2 - 1),
                            skip_group_check=True,
                            tile_position=(0, 0),
                        )
                # one PSUM->SBUF fp16 pass on the (otherwise idle) ACT
                # engine, folding the +sel64 in as the free per-partition
                # bias; releases st after this iteration (st bufs=2) even
                # though w consumes the scores 4 pieces later
                sc16 = work.tile([128, ch], F16, tag="sc16", bufs=6,
                                 name="sc16", padded_shape=[128, 1024])
                nc.scalar.activation(
                    sc16, st, mybir.ActivationFunctionType.Identity, bias=sel64
                )
            # stage C first: its inputs are 4 pieces old (surely ready), so
            # DVE opens the iteration with runnable work instead of blocking
            # on this piece's matmuls
            if 0 <= i - 4 < n_total:
                stage_c(i - 4, bag_off_c)
                bag_off_c += pieces[i - 4][1] // BAG
            if i < n_total:
                sm = work.tile([128, ch], F16, tag="sm", bufs=4, name="sm",
                               padded_shape=[128, 1024])
                nc.vector.tensor_mul(sm, st, oh_state[0][:, cs])
                pend_a[i] = (sc16, sm, ch)
            if 0 <= i - 2 < n_total:
                stage_b(i - 2)
    return nc


_NC_CACHE: dict = {}


def _get_nc(rows: int) -> bass.Bass:
    if rows not in _NC_CACHE:
        nc = build_nc(rows)
        nc.finalize()
        _NC_CACHE[rows] = nc
    return _NC_CACHE[rows]


def _numpy_fallback(x, rel_weight, bias, input_scope, query):
    """Pure-numpy replication of the reference for non-uniform bag layouts."""
    n = x.shape[0]
    num_bags = input_scope.shape[0] - 1
    seg = np.searchsorted(input_scope[1:], np.arange(n), side="right")
    att = np.einsum("nd,nd->n", x, rel_weight[query]).astype(np.float32)
    valid = seg < num_bags
    segv = seg[valid]
    attv = att[valid]
    m = np.full(num_bags, -np.inf, dtype=np.float32)
    np.maximum.at(m, segv, attv)
    e = np.zeros(n, dtype=np.float32)
    e[valid] = np.exp(attv - m[segv])
    z = np.zeros(num_bags, dtype=np.float32)
    np.add.at(z, segv, e[valid])
    w = np.zeros(n, dtype=np.float32)
    nz = z[segv] != 0
    w_valid = np.zeros(segv.shape[0], dtype=np.float32)
    w_valid[nz] = e[valid][nz] / z[segv][nz]
    w[valid] = w_valid
    repre = np.zeros((num_bags, x.shape[1]), dtype=np.float32)
    np.add.at(repre, segv, (x[valid] * w[valid][:, None]).astype(np.float32))
    return repre @ rel_weight.T + bias


def _pack_x(x_core):
    """[rows, D] fp32 -> [128, rows*KCH] fp16, superchunk-major with k-major
    runs inside each 1024-column superchunk so every superchunk DMA is one
    contiguous per-partition run of KCH*1024 elements."""
    rows = x_core.shape[0]
    xt = x_core.astype(np.float16).T                     # [D, rows]
    v = np.ascontiguousarray(xt).reshape(KCH, 128, rows // OH_GROUP, OH_GROUP)
    return np.ascontiguousarray(
        v.transpose(1, 2, 0, 3).reshape(128, rows * KCH)
    )


def _prepare_in_maps(x, rel_weight, bias, query):
    # block-1 (k=0..2) classes in columns 0:53 -> st rows 0:53;
    # block-2 (k=3..5) classes in columns 11:64 -> st rows 75:128, leaving
    # st row 64 zero for the z-selector trick
    rt = rel_weight.astype(np.float16).T.reshape(KCH, 128, C).transpose(1, 0, 2)
    relt = np.zeros((128, KCH, 64), dtype=np.float16)
    relt[:, : KCH // 2, :C] = rt[:, : KCH // 2, :]
    relt[:, KCH // 2 :, 11 : 11 + C] = rt[:, KCH // 2 :, :]
    sident = np.zeros((128, 128), dtype=np.float16)
    sident[np.arange(C), np.arange(C)] = 1.0
    sident[75 + np.arange(C), np.arange(C)] = 1.0
    # row 64 of lu is z, so a bias row folds bias*z into the recombine and
    # column 64 extracts z itself (for the host-side divide)
    sident[64, :C] = bias.astype(np.float16)
    sident[64, 64] = 1.0
    sel64 = np.zeros((128, 1), dtype=np.float16)
    sel64[64, 0] = 1.0
    q = query.astype(np.int64)
    in_maps = []
    for c in range(N_CORES):
        lo_r, hi_r = c * ROWS, (c + 1) * ROWS
        # fp8e4m3 one-hot built via its bit pattern (1.0 == 0x38)
        oh8 = np.zeros((128, ROWS), dtype=np.uint8)
        qc = q[lo_r:hi_r]
        ar = np.arange(ROWS)
        oh8[qc, ar] = 0x38
        oh8[75 + qc, ar] = 0x38
        oh = oh8.view(ml_dtypes.float8_e4m3)
        in_maps.append(
            {"xt2": _pack_x(x[lo_r:hi_r]), "oht": oh,
             "relt": relt, "sident": sident, "sel64": sel64}
        )
    return in_maps


def run_device(x, rel_weight, bias, query, trace=False, **kwargs):
    nc = _get_nc(ROWS)
    in_maps = _prepare_in_maps(x, rel_weight, bias, query)
    res = run_bass_kernel_spmd(
        nc, in_maps, core_ids=list(range(N_CORES)), trace=trace, **kwargs
    )
    outs = []
    for r in res.results:
        lt = np.asarray(r["out65"]).astype(np.float64)
        logits = (lt[0:C, :] / lt[64:65, :]).T.astype(np.float32)
        outs.append(np.ascontiguousarray(logits))
    return np.concatenate(outs, axis=0), res


def kernel(x, rel_weight, bias, input_scope, query):
    x = np.asarray(x, dtype=np.float32)
    rel_weight = np.asarray(rel_weight, dtype=np.float32)
    bias = np.asarray(bias, dtype=np.float32)
    input_scope = np.asarray(input_scope)
    query = np.asarray(query)

    expected_scope = np.arange(B + 1, dtype=np.int64) * (N // B)
    if (
        x.shape == (N, D)
        and rel_weight.shape == (C, D)
        and input_scope.shape == (B + 1,)
        and np.array_equal(input_scope.astype(np.int64), expected_scope)
    ):
        out, _ = run_device(x, rel_weight, bias, query)
        return out
    return _numpy_fallback(x, rel_weight, bias, input_scope, query)
